# revision 28
# baseline (speedup 1.0000x reference)
"""Trainium2 Bass kernel for nn_CrossAttention (B=2, N=2048, C=1024, H=16, D=64).

Strategy: sequence-parallel SPMD over 8 NeuronCores. Core i owns 512 rows of
the flattened [B*N, C] token axis (cores 0-3 = batch 0, cores 4-7 = batch 1).

Per core:
  - wave-0 DMA (x_s + K-half of W_kv) lands first; later DMA waves are gated
    purely by their position on the gpsimd FIFO behind the collective
    triggers, so they cannot steal bandwidth from the critical path
  - k^T/v AllGathers split into quarters and interleaved on the single CC
    stream (k0 v0 k1 k2 v1 k3 v2 v3) so attention starts ~25us after the
    stream opens and k/v quarters arrive just in time
  - AG outputs are loaded as per-rank contiguous blocks (the fused strided
    load runs at ~23 GB/s; contiguous 64-128KB blocks run at line rate)
  - attention in 2-head groups: S^T into double-buffered PSUM keeps the
    ScalarE exp chain back-to-back; P@V packs the two heads onto disjoint
    PE column groups; softmax row sums come from M=1 ones-matmuls
  - normalization is fully local: one reciprocal per pair, then K=1
    outer-product matmuls broadcast 1/rowsum across partitions into the
    spare PSUM bank; the multiply is deferred one pair so its dependency
    chain never blocks the PE/exp pipeline
  - fuse: out = a^T-chunks^T @ W_fuse + b_fuse
"""

import sys

if "/opt/trn_rl_repo" not in sys.path:
    sys.path.insert(0, "/opt/trn_rl_repo")

import numpy as np

B, N, C, H, D = 2, 2048, 1024, 16, 64
NCORES = 8
T = (B * N) // NCORES          # 512 tokens per core
P = 128
SCALE = D ** -0.5              # 0.125
QK = C * T // 4                # quarter of the k^T shard (2 m-tiles)
QV = T * C // 4                # quarter of the v shard (4 heads' dims)
GROUPS = [[0, 1, 2, 3], [4, 5, 6, 7]]

_CACHE = {}


def _build():
    import concourse.bass as bass
    import concourse.mybir as mybir
    import concourse.tile as tile
    from concourse import bacc
    from concourse.masks import make_identity

    f32 = mybir.dt.float32
    bf16 = mybir.dt.bfloat16

    nc = bacc.Bacc("TRN2", num_devices=NCORES, debug=False, enable_asserts=False)

    x_t = nc.dram_tensor("x_t", [T, C], f32, kind="ExternalInput").ap()
    x_s = nc.dram_tensor("x_s", [T, C], f32, kind="ExternalInput").ap()
    w_q = nc.dram_tensor("W_q", [C, C], f32, kind="ExternalInput").ap()
    w_kv = nc.dram_tensor("W_kv", [C, 2 * C], f32, kind="ExternalInput").ap()
    w_f = nc.dram_tensor("W_fuse", [C, C], f32, kind="ExternalInput").ap()
    b_f = nc.dram_tensor("b_fuse", [1, C], f32, kind="ExternalInput").ap()
    out = nc.dram_tensor("out", [T, C], f32, kind="ExternalOutput").ap()

    with tile.TileContext(nc) as tc:
        import contextlib

        with contextlib.ExitStack() as stk:
            consts = stk.enter_context(tc.tile_pool(name="consts", bufs=1))
            dram = stk.enter_context(tc.tile_pool(name="dram", bufs=1, space="DRAM"))

            identity = consts.tile([P, P], bf16, name="identity")
            make_identity(nc, identity)

            bias_b = consts.tile([P, C], bf16, name="bias_b")
            ones = consts.tile([P, D], bf16, name="ones")
            nc.gpsimd.memset(ones, 1.0)

            # persistent activations
            qT = [consts.tile([P, T], bf16, name=f"qT{m}") for m in range(8)]
            aT = [consts.tile([P, T], bf16, name=f"aT{g}") for g in range(8)]
            wf = [consts.tile([P, C], bf16, name=f"wf{c}") for c in range(8)]

            k_inQ = [dram.tile([QK], bf16, name=f"k_inQ{q}") for q in range(4)]
            v_inQ = [dram.tile([QV], bf16, name=f"v_inQ{q}") for q in range(4)]
            k_outQ = [dram.tile([4 * QK], bf16, name=f"k_outQ{q}") for q in range(4)]
            v_outQ = [dram.tile([4 * QV], bf16, name=f"v_outQ{q}") for q in range(4)]

            ptp = stk.enter_context(tc.tile_pool(name="ptp", bufs=24))
            sm = stk.enter_context(tc.tile_pool(name="sm", bufs=3))
            kfp = stk.enter_context(tc.tile_pool(name="kfp", bufs=1))
            kTf = [kfp.tile([P, 4 * T], bf16, name=f"kTf{m}") for m in range(8)]

            # ---- phase A: projections + interleaved AllGathers ----
            with tc.tile_pool(name="pa1", bufs=1) as pa, \
                 tc.tile_pool(name="pr_ps", bufs=3, space="PSUM") as pr_ps, \
                 tc.tile_pool(name="tp_ps", bufs=3, space="PSUM") as tp_ps:

                # wave 0 (t=0): x_s + K-half of W_kv only
                xs_nat = [pa.tile([P, C], bf16, name=f"xs_nat{i}") for i in range(4)]
                for i in range(4):
                    nc.gpsimd.dma_start(out=xs_nat[i], in_=x_s[i * P:(i + 1) * P, :])
                wkvK = [pa.tile([P, C], bf16, name=f"wkvK{c}") for c in range(8)]
                for c in range(8):
                    nc.gpsimd.dma_start(out=wkvK[c], in_=w_kv[c * P:(c + 1) * P, 0:C])

                xt_nat = [pa.tile([P, C], bf16, name=f"xt_nat{i}") for i in range(4)]
                wq = [pa.tile([P, C], bf16, name=f"wq{c}") for c in range(8)]
                wkvV = [pa.tile([P, C], bf16, name=f"wkvV{c}") for c in range(8)]

                # x_s transposes, c-major so xsT[c] completes early
                xsT = [pa.tile([P, T], bf16, name=f"xsT{c}") for c in range(8)]
                for c in range(8):
                    for i in range(4):
                        pst = tp_ps.tile([P, P], bf16, name="pst")
                        nc.tensor.transpose(
                            pst, xs_nat[i][:, c * P:(c + 1) * P], identity)
                        if c % 2 == 0:
                            nc.vector.tensor_copy(
                                out=xsT[c][:, i * P:(i + 1) * P], in_=pst)
                        else:
                            nc.scalar.copy(
                                out=xsT[c][:, i * P:(i + 1) * P], in_=pst)

                # k^T projection -> DRAM bounce; k quarter 0 fires after m=1,
                # then wave 1 rides the gpsimd FIFO behind the trigger
                k_inQ_v = [k_inQ[q].rearrange("(m p t) -> m p t", m=2, p=P, t=T)
                           for q in range(4)]
                for m in range(8):
                    ps = pr_ps.tile([P, T], f32, name="proj_ps")
                    for c in range(8):
                        nc.tensor.matmul(ps, wkvK[c][:, m * P:(m + 1) * P], xsT[c],
                                         start=(c == 0), stop=(c == 7))
                    kT_loc = pa.tile([P, T], bf16, name="kT_loc", bufs=3)
                    nc.vector.tensor_copy(out=kT_loc, in_=ps)
                    nc.sync.dma_start(out=k_inQ_v[m // 2][m % 2], in_=kT_loc)
                    if m == 1:
                        nc.gpsimd.collective_compute(
                            "AllGather", mybir.AluOpType.bypass,
                            replica_groups=GROUPS,
                            ins=[k_inQ[0][:].opt()], outs=[k_outQ[0][:].opt()])
                        for i in range(4):
                            nc.gpsimd.dma_start(
                                out=xt_nat[i], in_=x_t[i * P:(i + 1) * P, :])
                        for c in range(8):
                            nc.gpsimd.dma_start(
                                out=wq[c], in_=w_q[c * P:(c + 1) * P, :])
                        for c in range(8):
                            nc.gpsimd.dma_start(
                                out=wkvV[c], in_=w_kv[c * P:(c + 1) * P, C:2 * C])

                # x_t transposes now; the q projection goes LAST in phase A:
                # the first S^T waits (via PSUM bank WAR) for the final
                # phase-A drain, so make that drain the qT cast it needs anyway
                xtT = [pa.tile([P, T], bf16, name=f"xtT{c}") for c in range(8)]
                for c in range(8):
                    for i in range(4):
                        pst = tp_ps.tile([P, P], bf16, name="pst")
                        nc.tensor.transpose(
                            pst, xt_nat[i][:, c * P:(c + 1) * P], identity)
                        if c % 2 == 0:
                            nc.vector.tensor_copy(
                                out=xtT[c][:, i * P:(i + 1) * P], in_=pst)
                        else:
                            nc.scalar.copy(
                                out=xtT[c][:, i * P:(i + 1) * P], in_=pst)

                # v projection (natural layout, stored in dim quarters)
                v_inQ_v = [v_inQ[q].rearrange("(t p c) -> t p c", t=4, p=P, c=256)
                           for q in range(4)]
                for nh in range(2):
                    for tt in range(4):
                        ps = pr_ps.tile([P, T], f32, name="proj_ps")
                        for c in range(8):
                            nc.tensor.matmul(
                                ps,
                                xsT[c][:, tt * P:(tt + 1) * P],
                                wkvV[c][:, nh * 512:(nh + 1) * 512],
                                start=(c == 0), stop=(c == 7))
                        v_loc = pa.tile([P, 512], bf16, name="v_loc", bufs=4)
                        nc.vector.tensor_copy(out=v_loc, in_=ps)
                        nc.gpsimd.dma_start(
                            out=v_inQ_v[2 * nh][tt], in_=v_loc[:, 0:256])
                        nc.gpsimd.dma_start(
                            out=v_inQ_v[2 * nh + 1][tt], in_=v_loc[:, 256:512])

                for m in range(8):
                    ps = pr_ps.tile([P, T], f32, name="proj_ps")
                    for c in range(8):
                        nc.tensor.matmul(ps, wq[c][:, m * P:(m + 1) * P], xtT[c],
                                         start=(c == 0), stop=(c == 7))
                    nc.vector.tensor_copy(out=qT[m], in_=ps)

                # CC stream order: k0 (emitted above), v0, k1, k2, v1, k3, v2, v3
                def ag_k(q):
                    nc.gpsimd.collective_compute(
                        "AllGather", mybir.AluOpType.bypass, replica_groups=GROUPS,
                        ins=[k_inQ[q][:].opt()], outs=[k_outQ[q][:].opt()])

                def ag_v(q):
                    nc.gpsimd.collective_compute(
                        "AllGather", mybir.AluOpType.bypass, replica_groups=GROUPS,
                        ins=[v_inQ[q][:].opt()], outs=[v_outQ[q][:].opt()])

                ag_v(0); ag_k(1); ag_v(1); ag_k(2); ag_v(2); ag_k(3); ag_v(3)

            # ---------------- phase B: attention ----------------
            with tc.tile_pool(name="attn", bufs=1) as attn, \
                 tc.tile_pool(name="st_ps", bufs=2, space="PSUM") as st_ps, \
                 tc.tile_pool(name="ot_ps", bufs=1, space="PSUM") as ot_ps, \
                 tc.tile_pool(name="rs_ps", bufs=2, space="PSUM") as rs_ps, \
                 tc.tile_pool(name="rb_ps", bufs=1, space="PSUM") as rb_ps:

                # v tiles [128 keys, 16 kt, 256 dims] per head-quad quarter;
                # kTf tiles were allocated before phase A (no pool-reuse WAR)
                vpQ = [attn.tile([P, 16, 256], bf16, name=f"vpQ{q}")
                       for q in range(4)]
                kTf_v = [kTf[m].rearrange("p (r t) -> p r t", r=4) for m in range(8)]
                k_blk = [k_outQ[q].rearrange("(b p t) -> b p t", b=8, p=P, t=T)
                         for q in range(4)]
                v_blk = [v_outQ[q].rearrange("(b p c) -> b p c", b=16, p=P, c=256)
                         for q in range(4)]

                # sync is a pure load queue, ordered by CC stream arrival:
                # k0, v0, k1, k2, v1, k3, v2, v3
                def load_k(q):
                    for r in range(4):
                        for mm in range(2):
                            nc.sync.dma_start(
                                out=kTf_v[2 * q + mm][:, r, :],
                                in_=k_blk[q][r * 2 + mm])

                def load_v(q):
                    for kt in range(16):
                        nc.sync.dma_start(
                            out=vpQ[q][:, kt, :], in_=v_blk[q][kt])

                load_k(0); load_v(0); load_k(1); load_v(1)
                load_k(2); load_v(2); load_k(3); load_v(3)
                for c in range(8):
                    nc.gpsimd.dma_start(out=wf[c], in_=w_f[c * P:(c + 1) * P, :])
                nc.gpsimd.dma_start(out=bias_b, in_=b_f.to_broadcast([P, C]))

                def emit_st(g, kt):
                    # scores^T for heads 2g, 2g+1: row-packed, run concurrently
                    st = st_ps.tile([P, 2, T], f32, name="st")
                    for i in range(2):
                        nc.tensor.matmul(
                            st[:, i, :],
                            kTf[g][i * D:(i + 1) * D, kt * P:(kt + 1) * P],
                            qT[g][i * D:(i + 1) * D, :],
                            start=True, stop=True,
                            tile_position=(i * D, 0))
                    return st

                def emit_norm(g, rcb):
                    # broadcast 1/rowsum across partitions with K=1 matmuls
                    # into the spare PSUM bank, then scale aT[g]
                    rb = rb_ps.tile([P, T], f32, name="rb")
                    nc.tensor.matmul(rb[0:D, :], ones[0:1, :], rcb[0:1, :],
                                     start=True, stop=True,
                                     tile_position=(0, 0))
                    nc.tensor.matmul(rb[D:P, :], ones[32:33, :], rcb[32:33, :],
                                     start=True, stop=True,
                                     tile_position=(32, D))
                    nc.vector.tensor_mul(out=aT[g], in0=aT[g], in1=rb)

                pending = None
                st_next = emit_st(0, 0)
                LAG = 5                  # P@V trails exp by LAG chunks
                for g in range(8):           # head pairs
                    ot = ot_ps.tile([P, T], f32, name="ot")
                    rs = rs_ps.tile([33, T], f32, name="rs")
                    vp = vpQ[g // 2]
                    st = st_next
                    pts = []

                    def emit_pv(kt, g=g, ot=ot, rs=rs, vp=vp, pts=pts):
                        # P@V: the two heads on disjoint column groups
                        for i in range(2):
                            hh = (2 * g + i) % 4
                            nc.tensor.matmul(
                                ot[i * D:(i + 1) * D, :],
                                vp[:, kt, hh * D:(hh + 1) * D], pts[kt][:, i, :],
                                start=(kt == 0), stop=(kt == 15),
                                tile_position=(0, i * D))
                        # row sums via M=1 ones-matmuls on col groups 0/1
                        for i in range(2):
                            nc.tensor.matmul(
                                rs[i * 32:i * 32 + 1, :],
                                ones[:, 0:1], pts[kt][:, i, :],
                                start=(kt == 0), stop=(kt == 15),
                                tile_position=(0, i * 32))

                    for kt in range(16):     # key chunks of 128
                        pt = ptp.tile([P, 2, T], bf16, name="pt")
                        pts.append(pt)
                        nc.scalar.activation(
                            pt[:], st[:],
                            mybir.ActivationFunctionType.Exp, scale=SCALE)
                        # next chunk's scores issue on PE before any P@V so
                        # the exp chain never waits on the PE or v arrival
                        if kt < 15:
                            st = emit_st(g, kt + 1)
                            if kt == 14 and g < 7:
                                st_next = emit_st(g + 1, 0)
                        if kt == 6 and pending is not None:
                            emit_norm(*pending)
                            pending = None
                        if kt >= LAG:
                            emit_pv(kt - LAG)
                    for kt in range(16 - LAG, 16):
                        emit_pv(kt)
                    # drain: unnormalized O^T -> aT (bf16); one reciprocal over
                    # the whole rowsum bank (rows 0 and 32 are the real data)
                    nc.vector.tensor_copy(out=aT[g], in_=ot)
                    rcb = sm.tile([33, T], bf16, name="rcb")
                    with nc.allow_low_precision(reason="softmax 1/rowsum in bf16"):
                        nc.vector.reciprocal(rcb, rs)
                    pending = (g, rcb)
                emit_norm(*pending)

            # ---------------- phase C: fuse projection ----------------
            with tc.tile_pool(name="fu", bufs=4) as fu, \
                 tc.tile_pool(name="fu_ps", bufs=4, space="PSUM") as fu_ps:
                for tt in range(4):
                    for nh in range(2):
                        ps = fu_ps.tile([P, 512], f32, name="fps")
                        for c in range(8):
                            nc.tensor.matmul(
                                ps, aT[c][:, tt * P:(tt + 1) * P],
                                wf[c][:, nh * 512:(nh + 1) * 512],
                                start=(c == 0), stop=(c == 7))
                        ob = fu.tile([P, 512], f32, name="ob")
                        nc.vector.tensor_add(
                            out=ob, in0=ps, in1=bias_b[:, nh * 512:(nh + 1) * 512])
                        nc.sync.dma_start(
                            out=out[tt * P:(tt + 1) * P, nh * 512:(nh + 1) * 512],
                            in_=ob)

    nc.compile()
    return nc


def _get_nc():
    if "nc" not in _CACHE:
        _CACHE["nc"] = _build()
    return _CACHE["nc"]


def kernel(**inputs):
    nc = _get_nc()
    from concourse import bass_utils

    x_t = np.asarray(inputs["x_t"], dtype=np.float32).reshape(B * N, C)
    x_s = np.asarray(inputs["x_s"], dtype=np.float32).reshape(B * N, C)
    w_q = np.asarray(inputs["W_q"], dtype=np.float32)
    w_kv = np.asarray(inputs["W_kv"], dtype=np.float32)
    w_f = np.asarray(inputs["W_fuse"], dtype=np.float32)
    b_f = np.asarray(inputs["b_fuse"], dtype=np.float32).reshape(1, C)

    in_maps = []
    for i in range(NCORES):
        in_maps.append({
            "x_t": x_t[i * T:(i + 1) * T],
            "x_s": x_s[i * T:(i + 1) * T],
            "W_q": w_q,
            "W_kv": w_kv,
            "W_fuse": w_f,
            "b_fuse": b_f,
        })

    res = bass_utils.run_bass_kernel_spmd(nc, in_maps, core_ids=list(range(NCORES)))
    out = np.concatenate([res.results[i]["out"] for i in range(NCORES)], axis=0)
    return out.reshape(B, N, C).astype(np.float32)


if __name__ == "__main__":
    _build()
    print("build+compile OK")


# revision 32
# speedup vs baseline: 1.2368x; 1.2368x over previous
"""Trainium2 Bass kernel for nn_CrossAttention (B=2, N=2048, C=1024, H=16, D=64).

Strategy: sequence-parallel SPMD over 8 NeuronCores. Core i owns 512 rows of
the flattened [B*N, C] token axis (cores 0-3 = batch 0, cores 4-7 = batch 1).

Per core:
  - wave-0 DMA (x_s + K-half of W_kv) lands first; later DMA waves are gated
    purely by their position on the gpsimd FIFO behind the collective
    triggers, so they cannot steal bandwidth from the critical path
  - k^T/v AllGathers split into quarters and interleaved on the single CC
    stream (k0 v0 k1 k2 v1 k3 v2 v3) so attention starts ~25us after the
    stream opens and k/v quarters arrive just in time
  - AG outputs are loaded as per-rank contiguous blocks (the fused strided
    load runs at ~23 GB/s; contiguous 64-128KB blocks run at line rate)
  - attention in 2-head groups: S^T into double-buffered PSUM keeps the
    ScalarE exp chain back-to-back; P@V packs the two heads onto disjoint
    PE column groups; softmax row sums come from M=1 ones-matmuls
  - normalization is fully local: one reciprocal per pair, then K=1
    outer-product matmuls broadcast 1/rowsum across partitions into the
    spare PSUM bank; the multiply is deferred one pair so its dependency
    chain never blocks the PE/exp pipeline
  - fuse: out = a^T-chunks^T @ W_fuse + b_fuse
"""

import sys

if "/opt/trn_rl_repo" not in sys.path:
    sys.path.insert(0, "/opt/trn_rl_repo")

import numpy as np

B, N, C, H, D = 2, 2048, 1024, 16, 64
NCORES = 8
T = (B * N) // NCORES          # 512 tokens per core
P = 128
SCALE = D ** -0.5              # 0.125
QK = C * T // 4                # quarter of the k^T shard (2 m-tiles)
QV = T * C // 4                # quarter of the v shard (4 heads' dims)
GROUPS = [[0, 1, 2, 3], [4, 5, 6, 7]]

_CACHE = {}


def _build():
    import concourse.bass as bass
    import concourse.mybir as mybir
    import concourse.tile as tile
    from concourse import bacc
    from concourse.masks import make_identity

    f32 = mybir.dt.float32
    bf16 = mybir.dt.bfloat16

    nc = bacc.Bacc("TRN2", num_devices=NCORES, debug=False, enable_asserts=False)

    x_t = nc.dram_tensor("x_t", [T, C], f32, kind="ExternalInput").ap()
    x_s = nc.dram_tensor("x_s", [T, C], f32, kind="ExternalInput").ap()
    w_q = nc.dram_tensor("W_q", [C, C], f32, kind="ExternalInput").ap()
    w_kv = nc.dram_tensor("W_kv", [C, 2 * C], f32, kind="ExternalInput").ap()
    w_f = nc.dram_tensor("W_fuse", [C, C], f32, kind="ExternalInput").ap()
    b_f = nc.dram_tensor("b_fuse", [1, C], f32, kind="ExternalInput").ap()
    out = nc.dram_tensor("out", [T, C], f32, kind="ExternalOutput").ap()

    with tile.TileContext(nc) as tc:
        import contextlib

        with contextlib.ExitStack() as stk:
            consts = stk.enter_context(tc.tile_pool(name="consts", bufs=1))
            dram = stk.enter_context(tc.tile_pool(name="dram", bufs=1, space="DRAM"))

            identity = consts.tile([P, P], bf16, name="identity")
            make_identity(nc, identity)

            bias_b = consts.tile([P, C], bf16, name="bias_b")
            ones = consts.tile([P, D], bf16, name="ones")
            nc.gpsimd.memset(ones, 1.0)

            # persistent activations
            qT = [consts.tile([P, T], bf16, name=f"qT{m}") for m in range(8)]
            aT = [consts.tile([P, T], bf16, name=f"aT{g}") for g in range(8)]
            wf = [consts.tile([P, C], bf16, name=f"wf{c}") for c in range(8)]

            k_inQ = [dram.tile([QK], bf16, name=f"k_inQ{q}") for q in range(4)]
            v_inQ = [dram.tile([QV], bf16, name=f"v_inQ{q}") for q in range(4)]
            k_outQ = [dram.tile([4 * QK], bf16, name=f"k_outQ{q}") for q in range(4)]
            v_outQ = [dram.tile([4 * QV], bf16, name=f"v_outQ{q}") for q in range(4)]

            ptp = stk.enter_context(tc.tile_pool(name="ptp", bufs=24))
            sm = stk.enter_context(tc.tile_pool(name="sm", bufs=3))
            kfp = stk.enter_context(tc.tile_pool(name="kfp", bufs=1))
            kTf = [kfp.tile([P, 4 * T], bf16, name=f"kTf{m}") for m in range(8)]

            # ---- phase A: projections + interleaved AllGathers ----
            with tc.tile_pool(name="pa1", bufs=1) as pa, \
                 tc.tile_pool(name="pr_ps", bufs=3, space="PSUM") as pr_ps, \
                 tc.tile_pool(name="tp_ps", bufs=3, space="PSUM") as tp_ps:

                # wave 0 (t=0): x_s + K-half of W_kv only
                xs_nat = [pa.tile([P, C], bf16, name=f"xs_nat{i}") for i in range(4)]
                for i in range(4):
                    nc.gpsimd.dma_start(out=xs_nat[i], in_=x_s[i * P:(i + 1) * P, :])
                wkvK = [pa.tile([P, C], bf16, name=f"wkvK{c}") for c in range(8)]
                for c in range(8):
                    nc.gpsimd.dma_start(out=wkvK[c], in_=w_kv[c * P:(c + 1) * P, 0:C])

                xt_nat = [pa.tile([P, C], bf16, name=f"xt_nat{i}") for i in range(4)]
                wq = [pa.tile([P, C], bf16, name=f"wq{c}") for c in range(8)]
                wkvV = [pa.tile([P, C], bf16, name=f"wkvV{c}") for c in range(8)]

                # x_s transposes, c-major so xsT[c] completes early
                xsT = [pa.tile([P, T], bf16, name=f"xsT{c}") for c in range(8)]
                for c in range(8):
                    for i in range(4):
                        pst = tp_ps.tile([P, P], bf16, name="pst")
                        nc.tensor.transpose(
                            pst, xs_nat[i][:, c * P:(c + 1) * P], identity)
                        if c % 2 == 0:
                            nc.vector.tensor_copy(
                                out=xsT[c][:, i * P:(i + 1) * P], in_=pst)
                        else:
                            nc.scalar.copy(
                                out=xsT[c][:, i * P:(i + 1) * P], in_=pst)

                # k^T projection -> DRAM bounce; k quarter 0 fires after m=1,
                # then wave 1 rides the gpsimd FIFO behind the trigger
                k_inQ_v = [k_inQ[q].rearrange("(m p t) -> m p t", m=2, p=P, t=T)
                           for q in range(4)]
                for m in range(8):
                    ps = pr_ps.tile([P, T], f32, name="proj_ps")
                    for c in range(8):
                        nc.tensor.matmul(ps, wkvK[c][:, m * P:(m + 1) * P], xsT[c],
                                         start=(c == 0), stop=(c == 7))
                    kT_loc = pa.tile([P, T], bf16, name="kT_loc", bufs=3)
                    nc.vector.tensor_copy(out=kT_loc, in_=ps)
                    nc.sync.dma_start(out=k_inQ_v[m // 2][m % 2], in_=kT_loc)
                    if m == 1:
                        nc.gpsimd.collective_compute(
                            "AllGather", mybir.AluOpType.bypass,
                            replica_groups=GROUPS,
                            ins=[k_inQ[0][:].opt()], outs=[k_outQ[0][:].opt()])
                        for i in range(4):
                            nc.gpsimd.dma_start(
                                out=xt_nat[i], in_=x_t[i * P:(i + 1) * P, :])
                        for c in range(8):
                            nc.gpsimd.dma_start(
                                out=wq[c], in_=w_q[c * P:(c + 1) * P, :])
                        for c in range(8):
                            nc.gpsimd.dma_start(
                                out=wkvV[c], in_=w_kv[c * P:(c + 1) * P, C:2 * C])

                # x_t transposes + q^T projection BEFORE the v projection:
                # the first exp waits (via pool-reuse WAR) for the last
                # phase-A DVE op, so the q path must not be the straggler
                xtT = [pa.tile([P, T], bf16, name=f"xtT{c}") for c in range(8)]
                for c in range(8):
                    for i in range(4):
                        pst = tp_ps.tile([P, P], bf16, name="pst")
                        nc.tensor.transpose(
                            pst, xt_nat[i][:, c * P:(c + 1) * P], identity)
                        if c % 2 == 0:
                            nc.vector.tensor_copy(
                                out=xtT[c][:, i * P:(i + 1) * P], in_=pst)
                        else:
                            nc.scalar.copy(
                                out=xtT[c][:, i * P:(i + 1) * P], in_=pst)

                for m in range(8):
                    ps = pr_ps.tile([P, T], f32, name="proj_ps")
                    for c in range(8):
                        nc.tensor.matmul(ps, wq[c][:, m * P:(m + 1) * P], xtT[c],
                                         start=(c == 0), stop=(c == 7))
                    nc.vector.tensor_copy(out=qT[m], in_=ps)

                # v projection (natural layout, stored in dim quarters)
                v_inQ_v = [v_inQ[q].rearrange("(t p c) -> t p c", t=4, p=P, c=256)
                           for q in range(4)]
                for nh in range(2):
                    for tt in range(4):
                        ps = pr_ps.tile([P, T], f32, name="proj_ps")
                        for c in range(8):
                            nc.tensor.matmul(
                                ps,
                                xsT[c][:, tt * P:(tt + 1) * P],
                                wkvV[c][:, nh * 512:(nh + 1) * 512],
                                start=(c == 0), stop=(c == 7))
                        v_loc = pa.tile([P, 512], bf16, name="v_loc", bufs=4)
                        nc.vector.tensor_copy(out=v_loc, in_=ps)
                        nc.gpsimd.dma_start(
                            out=v_inQ_v[2 * nh][tt], in_=v_loc[:, 0:256])
                        nc.gpsimd.dma_start(
                            out=v_inQ_v[2 * nh + 1][tt], in_=v_loc[:, 256:512])

                # CC stream order: k0 (emitted above), v0, k1, k2, v1, k3, v2, v3
                def ag_k(q):
                    nc.gpsimd.collective_compute(
                        "AllGather", mybir.AluOpType.bypass, replica_groups=GROUPS,
                        ins=[k_inQ[q][:].opt()], outs=[k_outQ[q][:].opt()])

                def ag_v(q):
                    nc.gpsimd.collective_compute(
                        "AllGather", mybir.AluOpType.bypass, replica_groups=GROUPS,
                        ins=[v_inQ[q][:].opt()], outs=[v_outQ[q][:].opt()])

                ag_v(0); ag_k(1); ag_v(1); ag_k(2); ag_v(2); ag_k(3); ag_v(3)

            # ---------------- phase B: attention ----------------
            with tc.tile_pool(name="attn", bufs=1) as attn, \
                 tc.tile_pool(name="st_ps", bufs=2, space="PSUM") as st_ps, \
                 tc.tile_pool(name="ot_ps", bufs=1, space="PSUM") as ot_ps, \
                 tc.tile_pool(name="rs_ps", bufs=2, space="PSUM") as rs_ps, \
                 tc.tile_pool(name="rb_ps", bufs=1, space="PSUM") as rb_ps:

                # v tiles [128 keys, 16 kt, 256 dims] per head-quad quarter;
                # kTf tiles were allocated before phase A (no pool-reuse WAR)
                vpQ = [attn.tile([P, 16, 256], bf16, name=f"vpQ{q}")
                       for q in range(4)]
                kTf_v = [kTf[m].rearrange("p (r t) -> p r t", r=4) for m in range(8)]
                k_blk = [k_outQ[q].rearrange("(b p t) -> b p t", b=8, p=P, t=T)
                         for q in range(4)]
                v_blk = [v_outQ[q].rearrange("(b p c) -> b p c", b=16, p=P, c=256)
                         for q in range(4)]

                # sync is a pure load queue, ordered by CC stream arrival:
                # k0, v0, k1, k2, v1, k3, v2, v3
                def load_k(q):
                    for r in range(4):
                        for mm in range(2):
                            nc.sync.dma_start(
                                out=kTf_v[2 * q + mm][:, r, :],
                                in_=k_blk[q][r * 2 + mm])

                def load_v(q):
                    for kt in range(16):
                        nc.sync.dma_start(
                            out=vpQ[q][:, kt, :], in_=v_blk[q][kt])

                load_k(0); load_v(0); load_k(1); load_v(1)
                load_k(2); load_v(2); load_k(3); load_v(3)
                for c in range(8):
                    nc.gpsimd.dma_start(out=wf[c], in_=w_f[c * P:(c + 1) * P, :])
                nc.gpsimd.dma_start(out=bias_b, in_=b_f.to_broadcast([P, C]))

                def emit_st(g, kt):
                    # scores^T for heads 2g, 2g+1: row-packed, run concurrently
                    st = st_ps.tile([P, 2, T], f32, name="st")
                    for i in range(2):
                        nc.tensor.matmul(
                            st[:, i, :],
                            kTf[g][i * D:(i + 1) * D, kt * P:(kt + 1) * P],
                            qT[g][i * D:(i + 1) * D, :],
                            start=True, stop=True,
                            tile_position=(i * D, 0))
                    return st

                def emit_norm(g, rcb):
                    # broadcast 1/rowsum across partitions with K=1 matmuls
                    # into the spare PSUM bank, then scale aT[g]
                    rb = rb_ps.tile([P, T], f32, name="rb")
                    nc.tensor.matmul(rb[0:D, :], ones[0:1, :], rcb[0:1, :],
                                     start=True, stop=True,
                                     tile_position=(0, 0))
                    nc.tensor.matmul(rb[D:P, :], ones[32:33, :], rcb[32:33, :],
                                     start=True, stop=True,
                                     tile_position=(32, D))
                    nc.vector.tensor_mul(out=aT[g], in0=aT[g], in1=rb)

                pending = None
                prev_pv = prev_drain = None
                st_next = emit_st(0, 0)
                LAG = 5                  # P@V trails exp by LAG chunks
                for g in range(8):           # head pairs
                    ot = ot_ps.tile([P, T], f32, name="ot")
                    rs = rs_ps.tile([33, T], f32, name="rs")
                    vp = vpQ[g // 2]
                    st = st_next
                    pts = []

                    def emit_pv(kt, g=g, ot=ot, rs=rs, vp=vp, pts=pts):
                        # P@V: the two heads on disjoint column groups
                        for i in range(2):
                            hh = (2 * g + i) % 4
                            nc.tensor.matmul(
                                ot[i * D:(i + 1) * D, :],
                                vp[:, kt, hh * D:(hh + 1) * D], pts[kt][:, i, :],
                                start=(kt == 0), stop=(kt == 15),
                                tile_position=(0, i * D))
                        # row sums via M=1 ones-matmuls on col groups 0/1
                        for i in range(2):
                            nc.tensor.matmul(
                                rs[i * 32:i * 32 + 1, :],
                                ones[:, 0:1], pts[kt][:, i, :],
                                start=(kt == 0), stop=(kt == 15),
                                tile_position=(0, i * 32))

                    def drain(g=g, ot=ot, rs=rs):
                        # unnormalized O^T -> aT (bf16); one reciprocal over
                        # the whole rowsum bank (rows 0/32 are the real data)
                        nc.vector.tensor_copy(out=aT[g], in_=ot)
                        rcb = sm.tile([33, T], bf16, name="rcb")
                        with nc.allow_low_precision(
                                reason="softmax 1/rowsum in bf16"):
                            nc.vector.reciprocal(rcb, rs)
                        return (g, rcb)

                    for kt in range(16):     # key chunks of 128
                        pt = ptp.tile([P, 2, T], bf16, name="pt")
                        pts.append(pt)
                        nc.scalar.activation(
                            pt[:], st[:],
                            mybir.ActivationFunctionType.Exp, scale=SCALE)
                        # next chunk's scores issue on PE before any P@V so
                        # the exp chain never waits on the PE or v arrival
                        if kt < 15:
                            st = emit_st(g, kt + 1)
                            if kt == 14 and g < 7:
                                st_next = emit_st(g + 1, 0)
                        if kt == 10 and pending is not None:
                            emit_norm(*pending)
                            pending = None
                        # continuous lag: the previous pair's last P@Vs drain
                        # inside this pair's first chunks (no boundary burst)
                        if kt >= LAG:
                            emit_pv(kt - LAG)
                        elif prev_pv is not None:
                            prev_pv(16 - LAG + kt)
                            if kt == LAG - 1:
                                pending = prev_drain()
                    prev_pv, prev_drain = emit_pv, drain
                # flush the final pair
                for kt in range(16 - LAG, 16):
                    prev_pv(kt)
                pending = prev_drain()
                emit_norm(*pending)

            # ---------------- phase C: fuse projection ----------------
            with tc.tile_pool(name="fu", bufs=4) as fu, \
                 tc.tile_pool(name="fu_ps", bufs=4, space="PSUM") as fu_ps:
                for tt in range(4):
                    # both halves accumulate together so each stationary aT
                    # slice is loaded once for two matmuls
                    pss = [fu_ps.tile([P, 512], f32, name="fps") for _ in range(2)]
                    for c in range(8):
                        for nh in range(2):
                            nc.tensor.matmul(
                                pss[nh], aT[c][:, tt * P:(tt + 1) * P],
                                wf[c][:, nh * 512:(nh + 1) * 512],
                                start=(c == 0), stop=(c == 7))
                    for nh in range(2):
                        ob = fu.tile([P, 512], f32, name="ob")
                        nc.vector.tensor_add(
                            out=ob, in0=pss[nh],
                            in1=bias_b[:, nh * 512:(nh + 1) * 512])
                        nc.sync.dma_start(
                            out=out[tt * P:(tt + 1) * P, nh * 512:(nh + 1) * 512],
                            in_=ob)

    nc.compile()
    return nc


def _get_nc():
    if "nc" not in _CACHE:
        _CACHE["nc"] = _build()
    return _CACHE["nc"]


def kernel(**inputs):
    nc = _get_nc()
    from concourse import bass_utils

    x_t = np.asarray(inputs["x_t"], dtype=np.float32).reshape(B * N, C)
    x_s = np.asarray(inputs["x_s"], dtype=np.float32).reshape(B * N, C)
    w_q = np.asarray(inputs["W_q"], dtype=np.float32)
    w_kv = np.asarray(inputs["W_kv"], dtype=np.float32)
    w_f = np.asarray(inputs["W_fuse"], dtype=np.float32)
    b_f = np.asarray(inputs["b_fuse"], dtype=np.float32).reshape(1, C)

    in_maps = []
    for i in range(NCORES):
        in_maps.append({
            "x_t": x_t[i * T:(i + 1) * T],
            "x_s": x_s[i * T:(i + 1) * T],
            "W_q": w_q,
            "W_kv": w_kv,
            "W_fuse": w_f,
            "b_fuse": b_f,
        })

    res = bass_utils.run_bass_kernel_spmd(nc, in_maps, core_ids=list(range(NCORES)))
    out = np.concatenate([res.results[i]["out"] for i in range(NCORES)], axis=0)
    return out.reshape(B, N, C).astype(np.float32)


if __name__ == "__main__":
    _build()
    print("build+compile OK")
